# revision 1
# baseline (speedup 1.0000x reference)
"""nn_Encoder_48095043780825: 2-hop weighted-mean SAGEConv GNN encoder
on 8 Trainium2 NeuronCores (Bass/Tile), self-contained.

kernel(**inputs) -> np.ndarray [200000, 128] float32

Distribution (per-input JIT-specialized program; integer structure baked,
all float math on device):
 - Host relabels the 50k nodes, balanced by in-degree, into 8 cores x 49
   blocks of 128 slots (dst-sharding).  Edges are partitioned by destination
   block and packed into 128-edge chunks.
 - Per chunk, one indirect DMA gathers the 128 source rows; the DVE builds a
   mask M[e,dst] = w_e * (dst_local[e]==dst) from an iota tile; the PE
   accumulates agg += M.T @ msg in PSUM across the block's chunks.
 - Per block: normalize by 1/max(sum w,1e-12) (ACT per-partition scale),
   transpose, combine with the self path (W_self.T @ xT + W_neigh.T @ aggT),
   bias+ReLU on ACT, transpose back, DMA out.
 - AllGather of the new-x shards between hops (full table for next hop's
   gathers).
 - Final phase: queries sorted by effect id; per 128-query chunk an indirect
   gather of x2[node]; effect rows gathered once per group of chunks and
   expanded with a one-hot matmul; DVE add; host un-sorts the rows.
"""
import sys
sys.path.insert(0, "/opt/trn_rl_repo")
import heapq
import numpy as np

import jax
from jax.sharding import Mesh, PartitionSpec
from jax.experimental.shard_map import shard_map

from concourse import bass, mybir
from concourse.tile import TileContext
from concourse.bass2jax import (
    _bass_exec_p,
    install_neuronx_cc_hook,
    partition_id_tensor,
)

P = 128
F32 = mybir.dt.float32
I32 = mybir.dt.int32

CFG = dict(N=50000, E=600000, D=128, NEFF=1000, Q=200000, C=8, B=49, HOPS=2)


# ------------------------------------------------------------------ wait split

def _split_wide_waits(nc, max_waits=1):
    """This walrus build rejects instructions with more than one sync-wait
    command; move excess waits onto preceding NoOps on the same engine."""
    for f in nc.m.functions:
        for bb in f.blocks:
            new_instrs = []
            for ins in bb.instructions:
                si = ins.sync_info
                if si is not None and si.on_wait and len(si.on_wait) > max_waits:
                    waits = list(si.on_wait)
                    head, tail = waits[:-max_waits], waits[-max_waits:]
                    for i in range(0, len(head), max_waits):
                        nop = mybir.InstNoOp(
                            name=nc.get_next_instruction_name(),
                            engine=ins.engine,
                            ins=[], outs=[],
                            sync_info=mybir.SyncInfo(
                                on_wait=head[i:i + max_waits], on_update=[]),
                            text_hint="waitsplit",
                            bass_nofuse=True,
                        )
                        new_instrs.append(nop)
                    si.on_wait = tail
                new_instrs.append(ins)
            bb.instructions = new_instrs


# ------------------------------------------------------------------ host prep

def _balance_nodes(dst, N, n_bins):
    npad = n_bins * P
    deg = np.bincount(dst, minlength=N).astype(np.int64)
    deg_pad = np.zeros(npad, np.int64)
    deg_pad[:N] = deg
    order = np.argsort(-deg_pad, kind="stable")
    heap = [(0, b) for b in range(n_bins)]
    heapq.heapify(heap)
    counts = np.zeros(n_bins, np.int32)
    newid = np.empty(npad, np.int64)
    for n in order:
        while True:
            load, b = heapq.heappop(heap)
            if counts[b] < P:
                break
        newid[n] = b * P + counts[b]
        counts[b] += 1
        if counts[b] < P:
            heapq.heappush(heap, (load + deg_pad[n], b))
    assert counts.min() == counts.max() == P
    return newid


def _prep(inputs, cfg):
    N, E, D, NEFF, Q = cfg["N"], cfg["E"], cfg["D"], cfg["NEFF"], cfg["Q"]
    C, B = cfg["C"], cfg["B"]
    NPC = B * P
    NPAD = C * NPC
    assert NPAD >= N and D == 128

    graph_x = np.asarray(inputs["graph_x"], np.float32)
    edge_index = np.asarray(inputs["edge_index"])
    src = edge_index[0].astype(np.int64)
    dst = edge_index[1].astype(np.int64)
    w = np.asarray(inputs["chemical_similarity"], np.float32)
    x_nodes = np.asarray(inputs["x_nodes"]).astype(np.int64)
    effect_ids = np.asarray(inputs["effect_ids"]).astype(np.int64)
    W_self = np.asarray(inputs["W_self"], np.float32)
    W_neigh = np.asarray(inputs["W_neigh"], np.float32)
    bias = np.asarray(inputs["bias"], np.float32)
    effect_emb = np.asarray(inputs["effect_emb"], np.float32)

    newid = _balance_nodes(dst, N, C * B)

    x_full = np.zeros((NPAD, D), np.float32)
    x_full[newid[:N]] = graph_x

    nsrc = newid[src]
    ndst = newid[dst]
    ecore = ndst // NPC
    eblk = (ndst % NPC) // P
    eslot = ndst % P

    loads = np.zeros((C, B), np.int64)
    np.add.at(loads, (ecore, eblk), 1)
    C_b = np.maximum(1, -(-loads.max(axis=0) // P))
    NCH = int(C_b.sum())
    cob = np.concatenate([[0], np.cumsum(C_b)])

    MD = max(1, int(np.bincount(ndst, minlength=NPAD).max()))

    esrc = np.zeros((C, P, NCH), np.int32)
    edstf = np.zeros((C, P, NCH), np.float32)
    ewf = np.zeros((C, P, NCH), np.float32)
    wbd = np.zeros((C, P, B * MD), np.float32)

    eorder = np.lexsort((eslot, eblk, ecore))
    so, wo, slo = nsrc[eorder], w[eorder], eslot[eorder]
    idx = 0
    for c in range(C):
        for b in range(B):
            n = int(loads[c, b])
            sl = slice(idx, idx + n)
            idx += n
            base = cob[b] * P
            pos = base + np.arange(n)
            esrc[c, pos % P, pos // P] = so[sl]
            edstf[c, pos % P, pos // P] = slo[sl].astype(np.float32)
            ewf[c, pos % P, pos // P] = wo[sl]
            slot = slo[sl]
            korder = np.argsort(slot, kind="stable")
            ss, ww = slot[korder], wo[sl][korder]
            kth = np.zeros(P, np.int64)
            for s_, w_ in zip(ss, ww):
                wbd[c, s_, b * MD + kth[s_]] = w_
                kth[s_] += 1

    # queries
    QPC = -(-Q // C)
    QPAD = -(-QPC // P) * P
    QCH = QPAD // P
    qnode = np.zeros((C, P, QCH), np.int32)
    qorder = np.zeros((C, QPC), np.int64)
    GS = 16
    while True:
        ok = True
        for c in range(C):
            lo, hi = c * QPC, min((c + 1) * QPC, Q)
            es = np.sort(effect_ids[lo:hi])
            es_pad = np.concatenate(
                [es, np.full(QPAD - len(es), es[-1] if len(es) else 0)])
            for g0 in range(0, QCH, GS):
                if len(np.unique(es_pad[g0 * P:(g0 + GS) * P])) > P:
                    ok = False
                    break
            if not ok:
                break
        if ok:
            break
        GS //= 2
        assert GS >= 1
    NG = -(-QCH // GS)

    qeffd = np.zeros((C, P, NG), np.int32)
    slotmap = np.zeros((C, P, QCH), np.float32)
    for c in range(C):
        lo, hi = c * QPC, min((c + 1) * QPC, Q)
        nq = hi - lo
        eff_c = effect_ids[lo:hi]
        nod_c = x_nodes[lo:hi]
        o = np.argsort(eff_c, kind="stable")
        qorder[c, :nq] = o
        eff_s = eff_c[o]
        nod_s = newid[nod_c[o]]
        eff_pad = np.concatenate([eff_s, np.full(QPAD - nq, eff_s[-1] if nq else 0)])
        nod_pad = np.concatenate([nod_s, np.zeros(QPAD - nq, np.int64)])
        qnode[c] = nod_pad.reshape(QCH, P).T
        for g in range(NG):
            seg = eff_pad[g * GS * P:(g * GS + GS) * P]
            uniq = np.unique(seg)
            assert len(uniq) <= P
            qeffd[c, :len(uniq), g] = uniq
            smap = np.searchsorted(uniq, seg)
            s0, s1 = g * GS, min(g * GS + GS, QCH)
            slotmap[c, :, s0:s1] = smap[: (s1 - s0) * P].reshape(s1 - s0, P).T

    slotflat = np.transpose(slotmap, (0, 2, 1)).reshape(C, QCH * P).copy()

    iotaF = np.tile(np.arange(P, dtype=np.float32)[None, :], (P, 1))
    iotaP = np.tile(np.arange(P, dtype=np.float32)[:, None], (1, P))
    ident = np.eye(P, dtype=np.float32)

    meta = dict(cfg, NPC=NPC, NPAD=NPAD, NCH=NCH, MD=MD, QPC=QPC, QCH=QCH,
                GS=GS, NG=NG, C_b=C_b, chunk_of_block=cob, qorder=qorder)

    in_maps = []
    for c in range(C):
        in_maps.append({
            "x0_full": x_full,
            "x0_own": x_full[c * NPC:(c + 1) * NPC].copy(),
            "eff": effect_emb,
            "esrc": esrc[c], "edst": edstf[c], "ew": ewf[c], "wbd": wbd[c],
            "qnode": qnode[c], "qeffd": qeffd[c], "slotflat": slotflat[c],
            "iotaF": iotaF, "iotaP": iotaP, "ident": ident,
            "Wself": W_self, "Wneigh": W_neigh, "biasc": bias.T.copy(),
        })
    return meta, in_maps


# --------------------------------------------------------------- device build

def _build_nc(meta):
    C, B, D, NEFF = meta["C"], meta["B"], meta["D"], meta["NEFF"]
    NPC, NPAD, NCH, MD = meta["NPC"], meta["NPAD"], meta["NCH"], meta["MD"]
    QCH, GS, NG = meta["QCH"], meta["GS"], meta["NG"]
    C_b, cob = meta["C_b"], meta["chunk_of_block"]
    HOPS = meta["HOPS"]
    EPS = 1e-12

    nc = bass.Bass(trn_type="TRN2", num_devices=C, num_swdge_queues=2)

    x0_full = nc.dram_tensor("x0_full", [NPAD, D], F32, kind="ExternalInput")
    x0_own = nc.dram_tensor("x0_own", [NPC, D], F32, kind="ExternalInput")
    eff = nc.dram_tensor("eff", [NEFF, D], F32, kind="ExternalInput")
    esrc = nc.dram_tensor("esrc", [P, NCH], I32, kind="ExternalInput")
    edst = nc.dram_tensor("edst", [P, NCH], F32, kind="ExternalInput")
    ew = nc.dram_tensor("ew", [P, NCH], F32, kind="ExternalInput")
    wbd = nc.dram_tensor("wbd", [P, B * MD], F32, kind="ExternalInput")
    qnode = nc.dram_tensor("qnode", [P, QCH], I32, kind="ExternalInput")
    qeffd = nc.dram_tensor("qeffd", [P, NG], I32, kind="ExternalInput")
    slotflat = nc.dram_tensor("slotflat", [QCH * P], F32, kind="ExternalInput")
    iotaF = nc.dram_tensor("iotaF", [P, P], F32, kind="ExternalInput")
    iotaP = nc.dram_tensor("iotaP", [P, P], F32, kind="ExternalInput")
    ident = nc.dram_tensor("ident", [P, P], F32, kind="ExternalInput")
    Wself = nc.dram_tensor("Wself", [HOPS, D, D], F32, kind="ExternalInput")
    Wneigh = nc.dram_tensor("Wneigh", [HOPS, D, D], F32, kind="ExternalInput")
    biasc = nc.dram_tensor("biasc", [D, HOPS], F32, kind="ExternalInput")

    newx_loc = [nc.dram_tensor(f"newx{h}_loc", [NPC, D], F32) for h in range(HOPS)]
    x_shared = [nc.dram_tensor(f"x{h+1}_full", [NPAD, D], F32, addr_space="Shared")
                for h in range(HOPS)]
    out_dram = nc.dram_tensor("out", [QCH * P, D], F32, kind="ExternalOutput")

    rg = [list(range(C))]
    gq = [0]

    def gather(pool, table, off_ap, tag):
        t = pool.tile([P, D], F32, tag=tag, name=f"g_{tag}_{gq[0]}")
        bi = nc.gpsimd.indirect_dma_start(
            out=t[:], out_offset=None, in_=table[:],
            in_offset=bass.IndirectOffsetOnAxis(ap=off_ap, axis=0))
        if gq[0] % 2:
            bi.ins.queue = "qPoolDynamic1"
        gq[0] += 1
        return t

    with TileContext(nc) as tc:
        with tc.tile_pool(name="const", bufs=1) as cp:
            iF = cp.tile([P, P], F32)
            nc.sync.dma_start(out=iF[:], in_=iotaF[:, :])
            iP = cp.tile([P, P], F32)
            nc.sync.dma_start(out=iP[:], in_=iotaP[:, :])
            idn = cp.tile([P, P], F32)
            nc.sync.dma_start(out=idn[:], in_=ident[:, :])
            Ws, Wn = [], []
            for h in range(HOPS):
                t = cp.tile([P, D], F32, tag=f"ws{h}", name=f"ws{h}")
                nc.sync.dma_start(out=t[:], in_=Wself[h, :, :])
                Ws.append(t)
                t = cp.tile([P, D], F32, tag=f"wn{h}", name=f"wn{h}")
                nc.sync.dma_start(out=t[:], in_=Wneigh[h, :, :])
                Wn.append(t)
            bc = cp.tile([P, HOPS], F32)
            nc.sync.dma_start(out=bc[:], in_=biasc[:, :])
            edst_sb = cp.tile([P, NCH], F32)
            nc.sync.dma_start(out=edst_sb[:], in_=edst[:, :])
            ew_sb = cp.tile([P, NCH], F32)
            nc.sync.dma_start(out=ew_sb[:], in_=ew[:, :])
            esrc_sb = cp.tile([P, NCH], I32)
            nc.sync.dma_start(out=esrc_sb[:], in_=esrc[:, :])
            qnode_sb = cp.tile([P, QCH], I32)
            nc.sync.dma_start(out=qnode_sb[:], in_=qnode[:, :])
            qeffd_sb = cp.tile([P, NG], I32)
            nc.sync.dma_start(out=qeffd_sb[:], in_=qeffd[:, :])
            rden = cp.tile([P, B], F32)
            xT = [cp.tile([P, NPC], F32, tag=f"xT{i}", name=f"xT{i}")
                  for i in range(2)]

            with tc.tile_pool(name="den", bufs=4) as dp:
                wbd_sb = cp.tile([P, B * MD], F32)
                nc.sync.dma_start(out=wbd_sb[:], in_=wbd[:, :])
                for b in range(B):
                    dcol = dp.tile([P, 1], F32, tag="dcol")
                    nc.vector.reduce_sum(dcol[:], wbd_sb[:, b * MD:(b + 1) * MD],
                                         axis=mybir.AxisListType.X)
                    dmx = dp.tile([P, 1], F32, tag="dmx")
                    nc.vector.tensor_scalar_max(dmx[:], dcol[:], EPS)
                    nc.vector.reciprocal(rden[:, b:b + 1], dmx[:])

            with tc.tile_pool(name="xl", bufs=4) as xlp, \
                 tc.tile_pool(name="xlpp", bufs=2, space="PSUM") as xpp:
                for b in range(B):
                    xt = xlp.tile([P, P], F32, tag="xl")
                    nc.sync.dma_start(out=xt[:], in_=x0_own[b * P:(b + 1) * P, :])
                    ps = xpp.tile([P, P], F32, tag="xt", space="PSUM")
                    nc.tensor.transpose(out=ps[:], in_=xt[:], identity=idn[:])
                    nc.vector.tensor_copy(xT[0][:, b * P:(b + 1) * P], ps[:])

            for h in range(HOPS):
                table = x0_full if h == 0 else x_shared[h - 1]
                xT_cur, xT_nxt = xT[h % 2], xT[(h + 1) % 2]
                with tc.tile_pool(name=f"hop{h}", bufs=12) as hp, \
                     tc.tile_pool(name=f"hopb{h}", bufs=3) as hb, \
                     tc.tile_pool(name=f"hopp{h}", bufs=2, space="PSUM") as pp:
                    for b in range(B):
                        nchunks = int(C_b[b])
                        agg_ps = pp.tile([P, P], F32, tag="agg", space="PSUM")
                        for k in range(nchunks):
                            i = int(cob[b]) + k
                            msg = gather(hp, table, esrc_sb[:, i:i + 1], "msg")
                            mask = hp.tile([P, P], F32, tag="mask")
                            nc.vector.tensor_scalar(
                                out=mask[:], in0=iF[:],
                                scalar1=edst_sb[:, i:i + 1],
                                scalar2=ew_sb[:, i:i + 1],
                                op0=mybir.AluOpType.is_equal,
                                op1=mybir.AluOpType.mult)
                            nc.tensor.matmul(agg_ps[:], lhsT=mask[:], rhs=msg[:],
                                             start=(k == 0), stop=(k == nchunks - 1))
                        aggn = hb.tile([P, P], F32, tag="aggn")
                        nc.scalar.activation(aggn[:], agg_ps[:],
                                             mybir.ActivationFunctionType.Copy,
                                             scale=rden[:, b:b + 1])
                        aggT_ps = pp.tile([P, P], F32, tag="aggT", space="PSUM")
                        nc.tensor.transpose(out=aggT_ps[:], in_=aggn[:], identity=idn[:])
                        aggT = hb.tile([P, P], F32, tag="aggT_sb")
                        nc.vector.tensor_copy(aggT[:], aggT_ps[:])
                        hx_ps = pp.tile([P, P], F32, tag="hx", space="PSUM")
                        nc.tensor.matmul(hx_ps[:], lhsT=Ws[h][:],
                                         rhs=xT_cur[:, b * P:(b + 1) * P],
                                         start=True, stop=False)
                        nc.tensor.matmul(hx_ps[:], lhsT=Wn[h][:], rhs=aggT[:],
                                         start=False, stop=True)
                        nc.scalar.activation(xT_nxt[:, b * P:(b + 1) * P], hx_ps[:],
                                             mybir.ActivationFunctionType.Relu,
                                             bias=bc[:, h:h + 1])
                        nx_ps = pp.tile([P, P], F32, tag="nx", space="PSUM")
                        nc.tensor.transpose(out=nx_ps[:],
                                            in_=xT_nxt[:, b * P:(b + 1) * P],
                                            identity=idn[:])
                        nx = hb.tile([P, P], F32, tag="nx_sb")
                        nc.vector.tensor_copy(nx[:], nx_ps[:])
                        nc.sync.dma_start(out=newx_loc[h][b * P:(b + 1) * P, :],
                                          in_=nx[:])
                nc.gpsimd.collective_compute(
                    "AllGather", mybir.AluOpType.bypass,
                    replica_groups=rg,
                    ins=[newx_loc[h][:]],
                    outs=[x_shared[h][:]])

            with tc.tile_pool(name="fin", bufs=8) as fp, \
                 tc.tile_pool(name="fing", bufs=2) as fg, \
                 tc.tile_pool(name="finp", bufs=2, space="PSUM") as fpp:
                for g in range(NG):
                    EG = gather(fg, eff, qeffd_sb[:, g:g + 1], "EG")
                    smb = fg.tile([P, GS * P], F32, tag="smb")
                    c0 = g * GS
                    c1 = min(c0 + GS, QCH)
                    nc.sync.dma_start(
                        out=smb[:, :(c1 - c0) * P],
                        in_=slotflat[None, c0 * P:c1 * P].to_broadcast(
                            [P, (c1 - c0) * P]))
                    for j in range(c0, c1):
                        x2g = gather(fp, x_shared[HOPS - 1],
                                     qnode_sb[:, j:j + 1], "x2g")
                        oh = fp.tile([P, P], F32, tag="oh")
                        nc.vector.tensor_tensor(
                            out=oh[:], in0=iP[:],
                            in1=smb[:, (j - c0) * P:(j - c0 + 1) * P],
                            op=mybir.AluOpType.is_equal)
                        ef_ps = fpp.tile([P, P], F32, tag="efp", space="PSUM")
                        nc.tensor.matmul(ef_ps[:], lhsT=oh[:], rhs=EG[:],
                                         start=True, stop=True)
                        osb = fp.tile([P, P], F32, tag="osb")
                        nc.vector.tensor_add(osb[:], x2g[:], ef_ps[:])
                        nc.sync.dma_start(out=out_dram[j * P:(j + 1) * P, :],
                                          in_=osb[:])
    return nc


# ------------------------------------------------------------------- runner

def _build_runner(nc, n_cores):
    install_neuronx_cc_hook()
    partition_name = nc.partition_id_tensor.name if nc.partition_id_tensor else None

    in_names, out_names, out_avals = [], [], []
    for alloc in nc.m.functions[0].allocations:
        if not isinstance(alloc, mybir.MemoryLocationSet):
            continue
        name = alloc.memorylocations[0].name
        if alloc.kind == "ExternalInput":
            if name != partition_name:
                in_names.append(name)
        elif alloc.kind == "ExternalOutput":
            out_names.append(name)
            out_avals.append(jax.core.ShapedArray(
                tuple(alloc.tensor_shape), mybir.dt.np(alloc.dtype)))

    n_params = len(in_names)
    n_outs = len(out_avals)
    all_in_names = list(in_names) + list(out_names)
    if partition_name is not None:
        all_in_names.append(partition_name)

    def _body(*args):
        operands = list(args)
        if partition_name is not None:
            operands.append(partition_id_tensor())
        outs = _bass_exec_p.bind(
            *operands,
            out_avals=tuple(out_avals),
            in_names=tuple(all_in_names),
            out_names=tuple(out_names),
            lowering_input_output_aliases=(),
            sim_require_finite=True,
            sim_require_nnan=True,
            nc=nc,
        )
        return tuple(outs)

    devices = jax.devices()[:n_cores]
    mesh = Mesh(np.asarray(devices), ("core",))
    in_specs = (PartitionSpec("core"),) * (n_params + n_outs)
    out_specs = (PartitionSpec("core"),) * n_outs
    sharded = jax.jit(
        shard_map(_body, mesh=mesh, in_specs=in_specs, out_specs=out_specs,
                  check_rep=False),
        keep_unused=True,
    )

    def run(in_maps):
        per_core = [[np.asarray(m[name]) for name in in_names] for m in in_maps]
        concat_in = [
            np.concatenate([per_core[c][i] for c in range(n_cores)], axis=0)
            for i in range(n_params)
        ]
        concat_zeros = [
            np.zeros((n_cores * a.shape[0], *a.shape[1:]), a.dtype)
            for a in [np.zeros(av.shape, av.dtype) for av in out_avals]
        ]
        out_arrs = sharded(*concat_in, *concat_zeros)
        jax.block_until_ready(out_arrs)
        return [
            {name: np.asarray(out_arrs[i]).reshape(
                n_cores, *out_avals[i].shape)[c]
             for i, name in enumerate(out_names)}
            for c in range(n_cores)
        ]

    return run


# ------------------------------------------------------------------- kernel

def kernel(**inputs):
    gx = np.asarray(inputs["graph_x"])
    cfg = dict(
        N=gx.shape[0],
        E=np.asarray(inputs["edge_index"]).shape[1],
        D=gx.shape[1],
        NEFF=np.asarray(inputs["effect_emb"]).shape[0],
        Q=np.asarray(inputs["x_nodes"]).shape[0],
        C=8,
        B=-(-gx.shape[0] // (8 * P)),
        HOPS=np.asarray(inputs["W_self"]).shape[0],
    )
    meta, in_maps = _prep(inputs, cfg)
    nc = _build_nc(meta)
    _split_wide_waits(nc, 1)
    run = _build_runner(nc, cfg["C"])
    results = run(in_maps)

    C, QPC, D, Q = cfg["C"], meta["QPC"], cfg["D"], cfg["Q"]
    qorder = meta["qorder"]
    out = np.empty((Q, D), np.float32)
    for c in range(C):
        lo, hi = c * QPC, min((c + 1) * QPC, Q)
        nq = hi - lo
        out[lo + qorder[c, :nq]] = results[c]["out"][:nq]
    return out

